# revision 1
# baseline (speedup 1.0000x reference)
"""Trainium2 Bass kernel for nn_Attention_39676907884025.

Reference semantics: q_param (a scalar) is broadcast over both query and key,
so the score matrix qk[b,q,k] = sum_d p*p is CONSTANT along the softmax axis.
Softmax of a constant row is exactly uniform (x - max(x) == 0 bit-exactly,
exp(0) == 1, sum == SK exactly, 1/SK is a power of two), so

    out[b, q, :] = (1/SK) * sum_k value[b, k, :]     for every q.

query / key / q_param never need to touch the device.

Distribution: data-parallel over batch B=16 across 8 NeuronCores (2 batches
per core). Per core and batch (Tile-scheduled, measured 28.3us on HW):
  1. load value[b] (2048, 128) in four 256KB quarter-chunks, alternating the
     two HWDGE queues (Sync/Act); SBUF layout xt[p, t*128+d] = V[p*16+t, d]
     so each partition reads 16 consecutive DRAM rows (contiguous runs),
  2. exact fp32 DVE add-tree per quarter as it lands (overlaps later loads),
     then combine to acc (128, 128),
  3. one fp32 matmul with a constant (1/2048) matrix as stationary weights:
     psum[q, d] = sum_p (1/SK) * acc[p, d] -- reduces across partitions AND
     broadcasts the softmax-weighted mean row to all 128 q-partitions,
  4. replicate the tile 4x along free (doubling copies), then four 256KB
     stores (alternating queues) covering 4 output row-tiles each.
"""

import sys

import numpy as np

if "/opt/trn_rl_repo" not in sys.path:
    sys.path.insert(0, "/opt/trn_rl_repo")

B, SQ, SK, D, DV = 16, 2048, 2048, 128, 128
N_CORES = 8
BPC = B // N_CORES  # batches per core
P = 128

LAST_RESULT = None  # BassKernelResults of the most recent run (for profiling)


def _build_nc():
    import concourse.bacc as bacc
    import concourse.mybir as mybir
    from concourse.tile import TileContext

    f32 = mybir.dt.float32
    nc = bacc.Bacc("TRN2", target_bir_lowering=False)

    val = nc.dram_tensor("value", [BPC, SK, DV], f32, kind="ExternalInput")
    out = nc.dram_tensor("out", [BPC, SQ, DV], f32, kind="ExternalOutput")

    nt = SK // P  # 16 k-tiles per batch
    nq = SQ // P  # 16 q-tiles per batch

    with TileContext(nc) as tc:
        with (
            tc.tile_pool(name="x", bufs=3) as xpool,
            tc.tile_pool(name="tree", bufs=3) as tpool,
            tc.tile_pool(name="const", bufs=1) as cpool,
            tc.tile_pool(name="psum", bufs=4, space="PSUM") as ppool,
        ):
            # Stationary matmul operand: every entry 1/SK (exact power of 2),
            # so the partition-reduction matmul also applies the softmax
            # weight exactly.
            w = cpool.tile([P, P], f32)
            nc.vector.memset(w[:], 1.0 / SK)

            # Queue pick per chunk index: even -> Sync HWDGE, odd -> Act HWDGE.
            dma_eng = [nc.sync, nc.scalar]

            for b in range(BPC):
                # SBUF xt[p, t*128 + d] = value[b, p*16 + t, d]: each
                # partition owns 16 consecutive DRAM rows (8KB contiguous).
                # Load in 4 quarter-chunks (256KB each, alternating HWDGE
                # queues) and reduce each quarter as soon as it lands, so
                # DVE work overlaps the remaining loads.
                xt = xpool.tile([P, SK], f32)
                xdst = xt[:].rearrange("p (t d) -> p t d", d=DV)
                xsrc = val[b].rearrange("(p t) d -> p t d", p=P)

                accs = []
                for qi in range(4):
                    t0, t1 = 4 * qi, 4 * (qi + 1)
                    dma_eng[qi % 2].dma_start(
                        xdst[:, t0:t1, :], xsrc[:, t0:t1, :]
                    )
                    lo, hi = 512 * qi, 512 * (qi + 1)
                    a = tpool.tile([P, 256], f32, tag=f"a{qi % 2}")
                    nc.vector.tensor_add(
                        a[:], xt[:, lo : lo + 256], xt[:, lo + 256 : hi]
                    )
                    acc = tpool.tile([P, P], f32, tag=f"acc{qi}")
                    nc.vector.tensor_add(acc[:], a[:, 0:128], a[:, 128:256])
                    accs.append(acc)

                s01 = tpool.tile([P, P], f32, tag="s01")
                nc.vector.tensor_add(s01[:], accs[0][:], accs[1][:])
                s23 = tpool.tile([P, P], f32, tag="s23")
                nc.vector.tensor_add(s23[:], accs[2][:], accs[3][:])
                t4 = tpool.tile([P, P], f32, tag="t4")
                nc.vector.tensor_add(t4[:], s01[:], s23[:])

                # psum[q, d] = sum_p (1/SK) * t4[p, d]  for all q rows.
                ps = ppool.tile([P, P], f32)
                nc.tensor.matmul(ps[:], w[:], t4[:], start=True, stop=True)

                # Replicate the mean tile 4x along the free axis; each of the
                # 4 stores (alternating queues) reads it, covering 4 output
                # row-tiles (256KB) apiece.
                wide = xpool.tile([P, 512], f32, tag="wide")
                nc.vector.tensor_copy(wide[:, 0:P], ps[:])
                nc.vector.tensor_copy(wide[:, P : 2 * P], wide[:, 0:P])
                nc.vector.tensor_copy(wide[:, 2 * P : 4 * P], wide[:, 0 : 2 * P])

                odst = out[b].rearrange("(p t) d -> p t d", p=P)
                wsrc = wide[:].rearrange("p (t d) -> p t d", d=DV)
                for qi in range(4):
                    t0, t1 = 4 * qi, 4 * (qi + 1)
                    dma_eng[qi % 2].dma_start(odst[:, t0:t1, :], wsrc)

    nc.compile()
    return nc


def _build_nc_raw():
    """Raw bacc version (no TileContext): manual semaphores. Three DMA
    queues run continuously with balanced byte counts (~1.3MB each):

      sync   : b0 front loads (c0,c1) + b1c0    -> b0 front stores
      gpsimd : b0 back loads (c2,c3) + b1c1     -> b0 back + b1 back stores
      act    : b1 back loads (c2,c3) early (hides its ~3us cold-start lag)
                                                -> b1 front stores

    Each 256KB quarter is tree-reduced on DVE as it lands; one exact fp32
    (1/SK)-weighted matmul per batch does the partition reduction +
    broadcast; 4x replicate; 256KB stores.
    """
    import concourse.bacc as bacc
    import concourse.mybir as mybir

    f32 = mybir.dt.float32
    nc = bacc.Bacc("TRN2", target_bir_lowering=False)

    val = nc.dram_tensor("value", [BPC, SK, DV], f32, kind="ExternalInput")
    out = nc.dram_tensor("out", [BPC, SQ, DV], f32, kind="ExternalOutput")

    w = nc.alloc_sbuf_tensor("w_const", [P, P], f32)
    warm = nc.alloc_sbuf_tensor("warm", [P, DV], f32)
    xts = [nc.alloc_sbuf_tensor(f"xt{b}", [P, SK], f32) for b in range(BPC)]
    # scratch per batch: a-outs (4x256) in t1s, quarter accs (4x128) in t2s,
    # pair sums (2x128) in t3s, final acc in t4s
    t1s = [nc.alloc_sbuf_tensor(f"t1_{b}", [P, 1024], f32) for b in range(BPC)]
    t2s = [nc.alloc_sbuf_tensor(f"t2_{b}", [P, 512], f32) for b in range(BPC)]
    t3s = [nc.alloc_sbuf_tensor(f"t3_{b}", [P, 256], f32) for b in range(BPC)]
    t4s = [nc.alloc_sbuf_tensor(f"t4_{b}", [P, P], f32) for b in range(BPC)]
    wides = [nc.alloc_sbuf_tensor(f"wide{b}", [P, 512], f32) for b in range(BPC)]
    pss = [nc.alloc_psum_tensor(f"ps{b}", [P, P], f32) for b in range(BPC)]

    s_lq1 = nc.alloc_semaphore("s_lq1")  # sync loads
    s_lq2 = nc.alloc_semaphore("s_lq2")  # act loads
    s_lq3 = nc.alloc_semaphore("s_lq3")  # gpsimd loads
    s_w = nc.alloc_semaphore("s_w")
    s_dve = nc.alloc_semaphore("s_dve")
    s_mm = nc.alloc_semaphore("s_mm")
    s_wide = nc.alloc_semaphore("s_wide")
    s_ss = nc.alloc_semaphore("s_ss")  # sync stores
    s_sa = nc.alloc_semaphore("s_sa")  # act stores
    s_sg = nc.alloc_semaphore("s_sg")  # gpsimd stores
    s_warm = nc.alloc_semaphore("s_warm")

    def xdst(b):
        return xts[b][:].rearrange("p (t d) -> p t d", d=DV)

    def xsrc(b):
        return val[b].rearrange("(p t) d -> p t d", p=P)

    def odst(b):
        return out[b].rearrange("(p t) d -> p t d", p=P)

    def wsrc(b):
        return wides[b][:].rearrange("p (t d) -> p t d", d=DV)

    def load(eng, b, c, sem):
        return eng.dma_start(
            xdst(b)[:, 4 * c : 4 * c + 4, :], xsrc(b)[:, 4 * c : 4 * c + 4, :]
        ).then_inc(sem, 16)

    def store(eng, b, t0, sem):
        return eng.dma_start(odst(b)[:, t0 : t0 + 4, :], wsrc(b)).then_inc(
            sem, 16
        )

    with nc.Block() as block:

        @block.sync
        def _(sync):
            sync.dma_start(warm[0:1, :], val[0, 0:1, :]).then_inc(s_warm, 16)
            load(sync, 0, 0, s_lq1)  # s_lq1 = 16
            load(sync, 0, 1, s_lq1)  # 32
            load(sync, 1, 0, s_lq1)  # 48
            sync.wait_ge(s_wide, 1)
            store(sync, 0, 0, s_ss)
            store(sync, 0, 4, s_ss)
            sync.wait_ge(s_ss, 32)
            sync.wait_ge(s_warm, 16)

        @block.scalar
        def _(scalar):
            scalar.dma_start(warm[1:2, :], val[0, 1:2, :]).then_inc(s_warm, 16)
            load(scalar, 1, 2, s_lq2)  # s_lq2 = 16
            load(scalar, 1, 3, s_lq2)  # 32
            scalar.wait_ge(s_wide, 2)
            store(scalar, 1, 0, s_sa)
            store(scalar, 1, 4, s_sa)
            scalar.wait_ge(s_sa, 32)
            scalar.wait_ge(s_warm, 32)

        @block.gpsimd
        def _(gpsimd):
            load(gpsimd, 0, 2, s_lq3)  # s_lq3 = 16
            load(gpsimd, 0, 3, s_lq3)  # 32
            load(gpsimd, 1, 1, s_lq3)  # 48
            gpsimd.wait_ge(s_wide, 1)
            store(gpsimd, 0, 8, s_sg)
            store(gpsimd, 0, 12, s_sg)
            gpsimd.wait_ge(s_wide, 2)
            store(gpsimd, 1, 8, s_sg)
            store(gpsimd, 1, 12, s_sg)
            gpsimd.wait_ge(s_sg, 64)

        @block.vector
        def _(vector):
            vector.memset(w[:], 1.0 / SK).then_inc(s_w, 1)

            def qtree(b, c, sem, thresh):
                vector.wait_ge(sem, thresh)
                xt = xts[b]
                a = t1s[b][:, 256 * c : 256 * (c + 1)]
                vector.tensor_add(
                    a, xt[:, 512 * c : 512 * c + 256], xt[:, 512 * c + 256 : 512 * (c + 1)]
                )
                vector.tensor_add(
                    t2s[b][:, 128 * c : 128 * (c + 1)],
                    t1s[b][:, 256 * c : 256 * c + 128],
                    t1s[b][:, 256 * c + 128 : 256 * (c + 1)],
                )

            def combine(b):
                vector.tensor_add(
                    t3s[b][:, 0:128], t2s[b][:, 0:128], t2s[b][:, 128:256]
                )
                vector.tensor_add(
                    t3s[b][:, 128:256], t2s[b][:, 256:384], t2s[b][:, 384:512]
                )
                vector.tensor_add(
                    t4s[b][:], t3s[b][:, 0:128], t3s[b][:, 128:256]
                ).then_inc(s_dve, 1)

            def replicate(b):
                wide = wides[b]
                vector.wait_ge(s_mm, b + 1)
                vector.tensor_copy(wide[:, 0:P], pss[b][:])
                vector.tensor_copy(wide[:, P : 2 * P], wide[:, 0:P])
                vector.tensor_copy(
                    wide[:, 2 * P : 4 * P], wide[:, 0 : 2 * P]
                ).then_inc(s_wide, 1)

            # batch 0 quarters in expected landing order
            qtree(0, 0, s_lq1, 16)
            qtree(0, 2, s_lq3, 16)
            qtree(0, 1, s_lq1, 32)
            qtree(0, 3, s_lq3, 32)
            combine(0)
            # b1 back quarters (act queue, landed early) while mm0 runs
            qtree(1, 2, s_lq2, 16)
            qtree(1, 3, s_lq2, 32)
            replicate(0)
            qtree(1, 0, s_lq1, 48)
            qtree(1, 1, s_lq3, 48)
            combine(1)
            replicate(1)

        @block.tensor
        def _(tensor):
            tensor.wait_ge(s_w, 1)
            for b in range(BPC):
                tensor.wait_ge(s_dve, b + 1)
                nc.tensor.matmul(
                    pss[b][:], w[:], t4s[b][:], start=True, stop=True
                ).then_inc(s_mm, 1)

    nc.compile()
    return nc


KERNEL_VARIANT = "tile"  # "tile" or "raw"


def kernel(query=None, key=None, value=None, q_param=None, _trace=False):
    from concourse.bass_utils import run_bass_kernel_spmd

    global LAST_RESULT

    value = np.ascontiguousarray(np.asarray(value, dtype=np.float32))
    assert value.shape == (B, SK, DV), value.shape

    nc = _build_nc_raw() if KERNEL_VARIANT == "raw" else _build_nc()
    shards = value.reshape(N_CORES, BPC, SK, DV)
    in_maps = [{"value": shards[i]} for i in range(N_CORES)]

    LAST_RESULT = run_bass_kernel_spmd(
        nc, in_maps, list(range(N_CORES)), trace=_trace
    )
    return np.concatenate(
        [LAST_RESULT.results[i]["out"] for i in range(N_CORES)], axis=0
    )

